# revision 1
# baseline (speedup 1.0000x reference)
"""Trainium2 Bass kernel for nn_CrossAttention (B=16, C=128, N=128*128).

Sharding: data-parallel over batch across 8 cores (2 batches/core).

Per-batch decomposition (validated in fp64 against the reference):
  For each input side s in {0,1} (x_s -> q,k,v via 1x1 convs), raw = bias-free:
    psumA[n, 0:384]  = x_tile^T @ [wq^T|wk^T|wv^T]   (raw convs, transposed layout)
    psumA[n, 384]    = x^T (wq^T bq)                  (q bias cross term)
    psumA[n, 385]    = x^T (2 wk^T bk)                (k bias cross term, pre-doubled)
    sumsq_q[n] = ACTsq(q_raw) + 2*crossq + |bq|^2 ;  same for k
    rsk = 1/sqrt(sumsq_k);  K_s = k_raw * rsk  (normalized K, bias deferred)
    mat_mm[m,0:128] += K_s^T V_raw ; col 128 = A[m] = sum_n K_s[n,m]
    corr[0,:] = [B[c]=sum rsk*v_raw | sigma=sum rsk]; corr[1,:] = [Vs[c]=sum v_raw | .]
    mat_full = mat_mm + outer(e-EPS, bv) + outer(bk, B),  e = A + bk*sigma + EPS
    vsum = Vs + N*bv ;  q_nat[o,n] = wq @ x  (raw natural conv, 512-wide)
  Branch for side s (other = 1-s):
    P_s[n,0:128] = sq_s[n]*vsum_s[c] + mb_s[c] + q_s_nat^T mat_s   (mb = bq_s^T mat_s)
    P_s[n,128]   = q_s_nat^T e_other                               (den dot)
    den for branch OTHER... den_o' where: den(branch s) = 1/(N + rsq_o*(dot_o + bq_o.e_s))
    i.e. the dot computed in P_o's col 128 (q_o against e_s) feeds branch s's den.
    wvT_s[n,c] = P_s[:,0:128] * den_s * rsq_s
    out[o,n] = bout[o] + WV_0 @ wv_0^T + WV_1 @ wv_1^T   (att+cat convs fused)
"""

import numpy as np
import ml_dtypes

import concourse.bass as bass
from concourse import bacc
import concourse.mybir as mybir
import concourse.tile as tile
from concourse import bass_isa
from concourse.bass_utils import run_bass_kernel_spmd

BPC = 2          # batches per core
NCORES = 8
C = 128
N = 128 * 128
TS = 128         # n-tile size
NT = N // TS     # 128 tiles
CH = 512         # chunk width for x / q_nat convs
NCH = N // CH    # 32 chunks
TPC = CH // TS   # tiles per chunk = 4
EPS = 1e-6

F32 = mybir.dt.float32
F32R = mybir.dt.float32r
BF16 = mybir.dt.bfloat16
AF = mybir.ActivationFunctionType
ALU = mybir.AluOpType
BF = ml_dtypes.bfloat16


def build_nc(scal):
    """Build the Bass module. scal = dict of host scalar constants (betas)."""
    nc = bacc.Bacc("TRN2", target_bir_lowering=False)

    x1_d = nc.dram_tensor("x1", [BPC, C, N], F32, kind="ExternalInput")
    x2_d = nc.dram_tensor("x2", [BPC, C, N], F32, kind="ExternalInput")
    out_d = nc.dram_tensor("out", [BPC, C, N], F32, kind="ExternalOutput")
    wqkv1_d = nc.dram_tensor("wqkv1", [C, 386], BF16, kind="ExternalInput")
    wqkv2_d = nc.dram_tensor("wqkv2", [C, 386], BF16, kind="ExternalInput")
    wqT1_d = nc.dram_tensor("wqT1", [C, C], BF16, kind="ExternalInput")
    wqT2_d = nc.dram_tensor("wqT2", [C, C], BF16, kind="ExternalInput")
    WV1T_d = nc.dram_tensor("WV1T", [C, C], BF16, kind="ExternalInput")
    WV2T_d = nc.dram_tensor("WV2T", [C, C], BF16, kind="ExternalInput")
    ident_d = nc.dram_tensor("ident", [C, C], BF16, kind="ExternalInput")
    onesrow_d = nc.dram_tensor("onesrow", [1, C], BF16, kind="ExternalInput")
    boutrow_d = nc.dram_tensor("boutrow", [1, C], BF16, kind="ExternalInput")
    bqb_d = nc.dram_tensor("bqb", [C, 2], BF16, kind="ExternalInput")
    bcolsf_d = nc.dram_tensor("bcolsf", [C, 4], F32, kind="ExternalInput")
    bvN_d = nc.dram_tensor("bvN", [1, 2 * C], F32, kind="ExternalInput")
    bkrows_d = nc.dram_tensor("bkrows", [2, C], BF16, kind="ExternalInput")
    bvrows_d = nc.dram_tensor("bvrows", [2, C], BF16, kind="ExternalInput")

    with tile.TileContext(nc) as tc:
        with (
            tc.tile_pool(name="consts", bufs=1) as consts,
            tc.tile_pool(name="xch", bufs=2) as xch,
            tc.tile_pool(name="kv", bufs=3) as kv,
            tc.tile_pool(name="qnat", bufs=1) as qnatp,
            tc.tile_pool(name="stats", bufs=1) as stats,
            tc.tile_pool(name="cols", bufs=6) as colsp,
            tc.tile_pool(name="wvp", bufs=2) as wvp,
            tc.tile_pool(name="osb", bufs=3) as osbp,
            tc.tile_pool(name="junk", bufs=2) as junkp,
            tc.tile_pool(name="pA", bufs=2, space="PSUM") as pA,
            tc.tile_pool(name="pB", bufs=2, space="PSUM") as pB,
            tc.tile_pool(name="pW", bufs=2, space="PSUM") as pW,
            tc.tile_pool(name="pM", bufs=1, space="PSUM") as pM,
        ):
            # ---- constants ----
            wqkv = [consts.tile([C, 386], BF16, tag=f"wqkv{s}", name=f"wqkv{s}")
                    for s in range(2)]
            nc.sync.dma_start(out=wqkv[0], in_=wqkv1_d[:, :])
            nc.sync.dma_start(out=wqkv[1], in_=wqkv2_d[:, :])
            wqT = [consts.tile([C, C], BF16, tag=f"wqT{s}", name=f"wqT{s}")
                   for s in range(2)]
            nc.sync.dma_start(out=wqT[0], in_=wqT1_d[:, :])
            nc.sync.dma_start(out=wqT[1], in_=wqT2_d[:, :])
            WVT = [consts.tile([C, C], BF16, tag=f"WVT{s}", name=f"WVT{s}")
                   for s in range(2)]
            nc.sync.dma_start(out=WVT[0], in_=WV1T_d[:, :])
            nc.sync.dma_start(out=WVT[1], in_=WV2T_d[:, :])
            ident = consts.tile([C, C], BF16, tag="ident")
            nc.sync.dma_start(out=ident, in_=ident_d[:, :])
            onesrow = consts.tile([1, C], BF16, tag="onesrow")
            nc.sync.dma_start(out=onesrow, in_=onesrow_d[:, :])
            boutrow = consts.tile([1, C], BF16, tag="boutrow")
            nc.sync.dma_start(out=boutrow, in_=boutrow_d[:, :])
            bqb = consts.tile([C, 2], BF16, tag="bqb")
            nc.sync.dma_start(out=bqb, in_=bqb_d[:, :])
            bcolsf = consts.tile([C, 4], F32, tag="bcolsf")
            nc.sync.dma_start(out=bcolsf, in_=bcolsf_d[:, :])
            bvN = consts.tile([1, 2 * C], F32, tag="bvN")
            nc.sync.dma_start(out=bvN, in_=bvN_d[:, :])
            eM = [consts.tile([2, C], BF16, tag=f"eM{s}", name=f"eM{s}")
                  for s in range(2)]
            nc.sync.dma_start(out=eM[0][1:2, :], in_=bkrows_d[0:1, :])
            nc.sync.dma_start(out=eM[1][1:2, :], in_=bkrows_d[1:2, :])
            corrRHS = [consts.tile([2, C], BF16, tag=f"cR{s}", name=f"cR{s}")
                       for s in range(2)]
            nc.sync.dma_start(out=corrRHS[0][0:1, :], in_=bvrows_d[0:1, :])
            nc.sync.dma_start(out=corrRHS[1][0:1, :], in_=bvrows_d[1:2, :])

            xs_d = [x1_d, x2_d]
            betas_q = [scal["bq1sq"], scal["bq2sq"]]
            betas_k = [scal["bk1sq"], scal["bk2sq"]]

            for b in range(BPC):
                # ---------------- PASS 1 ----------------
                matcomb = pM.tile([C, 512], F32, tag="matcomb")
                corrcomb = pM.tile([4, 512], F32, tag="corrcomb")
                nqall = [stats.tile([C, NT], F32, tag=f"nqall{s}", name=f"nqall{s}")
                         for s in range(2)]
                cqall = [stats.tile([C, NT], F32, tag=f"cqall{s}", name=f"cqall{s}")
                         for s in range(2)]
                qnat = [qnatp.tile([C, N], BF16, tag=f"qnat{s}", name=f"qnat{s}")
                        for s in range(2)]

                for ch in range(NCH):
                    xc = [xch.tile([C, CH], F32, tag=f"xc{s}", name=f"xc{s}")
                          for s in range(2)]
                    xb = [xch.tile([C, CH], BF16, tag=f"xb{s}", name=f"xb{s}")
                          for s in range(2)]
                    for s in range(2):
                        nc.gpsimd.dma_start(
                            out=xc[s], in_=xs_d[s][b, :, ch * CH:(ch + 1) * CH])
                        nc.gpsimd.tensor_copy(out=xb[s], in_=xc[s])
                    for s in range(2):
                        pq = pB.tile([C, CH], F32, tag="pb", name="pqnat")
                        nc.tensor.matmul(pq, wqT[s], xb[s], start=True, stop=True)
                        nc.scalar.copy(out=qnat[s][:, ch * CH:(ch + 1) * CH], in_=pq)
                    for t in range(TPC):
                        i = ch * TPC + t
                        for s in range(2):
                            moff = 256 * s
                            psA = pA.tile([TS, 386], F32, tag="ps", name="psA")
                            nc.tensor.matmul(
                                psA, xb[s][:, t * TS:(t + 1) * TS], wqkv[s],
                                start=True, stop=True)
                            ja = junkp.tile([TS, C], BF16, tag="junkA", name="ja")
                            nc.scalar.activation(
                                out=ja, in_=psA[:, 0:128], func=AF.Square,
                                accum_out=nqall[s][:, i:i + 1])
                            nkcol = colsp.tile([TS, 1], F32, tag="nk", name="nkcol")
                            jb = junkp.tile([TS, C], BF16, tag="junkB", name="jb")
                            nc.scalar.activation(
                                out=jb, in_=psA[:, 128:256], func=AF.Square,
                                accum_out=nkcol)
                            nc.vector.tensor_copy(
                                out=cqall[s][:, i:i + 1], in_=psA[:, 384:385])
                            tcol = colsp.tile([TS, 1], F32, tag="tk", name="tcol")
                            nc.vector.tensor_scalar(
                                out=tcol, in0=psA[:, 385:386],
                                scalar1=float(betas_k[s]), scalar2=None, op0=ALU.add)
                            skcol = colsp.tile([TS, 1], F32, tag="sk", name="skcol")
                            nc.scalar.activation(
                                out=skcol, in_=nkcol, func=AF.Sqrt,
                                bias=tcol, scale=1.0)
                            rskcol = colsp.tile([TS, 1], F32, tag="rsk", name="rskcol")
                            nc.vector.reciprocal(out=rskcol, in_=skcol)
                            cl = kv.tile([TS, 2], BF16, tag=f"cl{s}", name=f"cl{s}")
                            nc.scalar.copy(out=cl[:, 0:1], in_=rskcol)
                            nc.vector.memset(cl[:, 1:2], 1.0)
                            ks = kv.tile([TS, C], BF16, tag=f"k{s}", name=f"ks{s}")
                            nc.vector.tensor_scalar(
                                out=ks, in0=psA[:, 128:256],
                                scalar1=rskcol, scalar2=None, op0=ALU.mult)
                            vs = kv.tile([TS, C + 4], BF16, tag=f"v{s}", name=f"vs{s}")
                            nc.scalar.copy(out=vs[:, 0:128], in_=psA[:, 256:384])
                            nc.vector.memset(vs[:, 128:129], 1.0)
                            nc.tensor.matmul(
                                matcomb[:, moff:moff + 129], ks, vs[:, 0:129],
                                start=(i == 0), stop=False, skip_group_check=True)
                            nc.tensor.matmul(
                                corrcomb[0:2, moff:moff + 129], cl, vs[:, 0:129],
                                start=(i == 0), stop=(i == NT - 1),
                                skip_group_check=True)

                # ---------------- PASS 1.5 ----------------
                rsqall = [stats.tile([C, NT], F32, tag=f"rsqall{s}", name=f"rsqall{s}")
                          for s in range(2)]
                sqallT = [stats.tile([C, NT], BF16, tag=f"sqT{s}", name=f"sqT{s}")
                          for s in range(2)]
                sqrow = [stats.tile([1, N], BF16, tag=f"sqrow{s}", name=f"sqrow{s}")
                         for s in range(2)]
                matR = [stats.tile([C, 132], BF16, tag=f"matR{s}", name=f"matR{s}")
                        for s in range(2)]
                vsumrowX = [stats.tile([1, 132], BF16, tag=f"vsX{s}", name=f"vsX{s}")
                            for s in range(2)]
                mbrowX = [stats.tile([1, 132], BF16, tag=f"mbX{s}", name=f"mbX{s}")
                          for s in range(2)]
                udenall = [None, None]
                ecolf = [None, None]
                ecolb = [None, None]
                for s in range(2):
                    moff = 256 * s
                    tq = stats.tile([C, NT], F32, tag=f"tq{s}", name=f"tq{s}")
                    nc.vector.tensor_scalar(
                        out=tq, in0=cqall[s], scalar1=2.0,
                        scalar2=float(betas_q[s]), op0=ALU.mult, op1=ALU.add)
                    nc.vector.tensor_tensor(out=tq, in0=tq, in1=nqall[s], op=ALU.add)
                    sqf = stats.tile([C, NT], F32, tag=f"sqf{s}", name=f"sqf{s}")
                    nc.scalar.activation(out=sqf, in_=tq, func=AF.Sqrt)
                    nc.vector.reciprocal(out=rsqall[s], in_=sqf)
                    sqb = stats.tile([C, NT], BF16, tag=f"sqb{s}", name=f"sqb{s}")
                    nc.vector.tensor_copy(out=sqb, in_=sqf)
                    pt = pW.tile([C, NT], BF16, tag="pw", name="ptr")
                    nc.tensor.transpose(pt, sqb, ident)
                    nc.scalar.copy(out=sqallT[s], in_=pt)
                    # flatten [tile, n] rows into one partition so pass-2 outer
                    # matmuls can slice lhsT at base partition 0
                    nc.sync.dma_start(out=sqrow[s], in_=sqallT[s])
                    # e col
                    sig1 = stats.tile([1, 1], F32, tag=f"sg{s}", name=f"sg{s}")
                    nc.scalar.copy(out=sig1, in_=corrcomb[0:1, moff + 128:moff + 129])
                    sigc = stats.tile([C, 1], F32, tag=f"sgc{s}", name=f"sgc{s}")
                    nc.gpsimd.partition_broadcast(sigc, sig1)
                    e0 = stats.tile([C, 1], F32, tag=f"e0{s}", name=f"e0{s}")
                    nc.vector.tensor_scalar(
                        out=e0, in0=bcolsf[:, 2 * s + 1:2 * s + 2],
                        scalar1=sigc, scalar2=None, op0=ALU.mult)
                    nc.vector.tensor_tensor(
                        out=e0, in0=e0, in1=matcomb[:, moff + 128:moff + 129],
                        op=ALU.add)
                    ecol = stats.tile([C, 1], F32, tag=f"ec{s}", name=f"ec{s}")
                    nc.vector.tensor_scalar(
                        out=ecol, in0=e0, scalar1=EPS, scalar2=None, op0=ALU.add)
                    ecolf[s] = ecol
                    eb = stats.tile([C, 1], BF16, tag=f"ecb{s}", name=f"ecb{s}")
                    nc.vector.tensor_copy(out=eb, in_=ecol)
                    ecolb[s] = eb
                    e0b = stats.tile([C, 1], BF16, tag=f"e0b{s}", name=f"e0b{s}")
                    nc.vector.tensor_copy(out=e0b, in_=e0)
                    pe = pW.tile([1, C], BF16, tag="pw", name="per")
                    nc.tensor.transpose(pe, e0b, ident)
                    nc.scalar.copy(out=eM[s][0:1, :], in_=pe)
                    Bb = stats.tile([1, C], BF16, tag=f"Bb{s}", name=f"Bb{s}")
                    nc.vector.tensor_copy(out=Bb, in_=corrcomb[0:1, moff:moff + 128])
                    nc.sync.dma_start(out=corrRHS[s][1:2, :], in_=Bb)
                    nc.tensor.matmul(
                        matcomb[:, moff:moff + 128], eM[s], corrRHS[s],
                        start=False, stop=True, skip_group_check=True)
                    nc.vector.tensor_copy(out=matR[s][:, 0:128],
                                          in_=matcomb[:, moff:moff + 128])
                    corrsb = stats.tile([2, C], F32, tag=f"csb{s}", name=f"csb{s}")
                    nc.scalar.copy(out=corrsb, in_=corrcomb[0:2, moff:moff + 128])
                    vsr = stats.tile([1, C], F32, tag=f"vsr{s}", name=f"vsr{s}")
                    nc.sync.dma_start(out=vsr, in_=corrsb[1:2, :])
                    vsf = stats.tile([1, C], F32, tag=f"vsf{s}", name=f"vsf{s}")
                    nc.vector.tensor_tensor(
                        out=vsf, in0=vsr, in1=bvN[0:1, s * C:(s + 1) * C], op=ALU.add)
                    nc.vector.memset(vsumrowX[s][:, 128:132], 0.0)
                    nc.vector.tensor_copy(out=vsumrowX[s][:, 0:128], in_=vsf)
                for s in range(2):
                    o = 1 - s
                    nc.vector.tensor_copy(out=matR[s][:, 128:129], in_=ecolb[o])
                    bqe = stats.tile([C, 1], F32, tag=f"bqe{s}", name=f"bqe{s}")
                    nc.vector.tensor_tensor(
                        out=bqe, in0=bcolsf[:, 2 * s:2 * s + 1], in1=ecolf[o],
                        op=ALU.mult)
                    bqec = stats.tile([C, 1], F32, tag=f"bqec{s}", name=f"bqec{s}")
                    nc.gpsimd.partition_all_reduce(
                        bqec, bqe, channels=C, reduce_op=bass_isa.ReduceOp.add)
                    ud = stats.tile([C, NT], F32, tag=f"uden{s}", name=f"uden{s}")
                    nc.vector.tensor_scalar(
                        out=ud, in0=rsqall[s], scalar1=bqec, scalar2=float(N),
                        op0=ALU.mult, op1=ALU.add)
                    udenall[s] = ud
                    pm = pA.tile([1, C], F32, tag="ps", name="pmb")
                    nc.tensor.matmul(pm, bqb[:, s:s + 1], matR[s][:, 0:128],
                                     start=True, stop=True)
                    nc.vector.memset(mbrowX[s][:, 128:132], 0.0)
                    nc.vector.tensor_copy(out=mbrowX[s][:, 0:128], in_=pm)

                # ---------------- PASS 2 ----------------
                for i in range(NT):
                    psP = [None, None]
                    for s in range(2):
                        psP[s] = pA.tile([TS, 132], F32, tag="ps", name="psP")
                        nc.tensor.matmul(
                            psP[s][:, 0:129], sqrow[s][0:1, i * TS:(i + 1) * TS],
                            vsumrowX[s][:, 0:129],
                            start=True, stop=False, skip_group_check=True)
                        nc.tensor.matmul(
                            psP[s][:, 0:129], onesrow, mbrowX[s][:, 0:129],
                            start=False, stop=False, skip_group_check=True)
                        nc.tensor.matmul(
                            psP[s][:, 0:129], qnat[s][:, i * TS:(i + 1) * TS],
                            matR[s][:, 0:129],
                            start=False, stop=True, skip_group_check=True)
                    wvn = [None, None]
                    for s in range(2):
                        o = 1 - s
                        # dot in psP[s][:,128] pairs with branch o's den
                        dcol = colsp.tile([TS, 1], F32, tag="dt", name="dcol")
                        nc.vector.tensor_scalar(
                            out=dcol, in0=psP[s][:, 128:129],
                            scalar1=rsqall[s][:, i:i + 1],
                            scalar2=udenall[s][:, i:i + 1],
                            op0=ALU.mult, op1=ALU.add)
                        deno = colsp.tile([TS, 1], F32, tag="den", name="deno")
                        nc.vector.reciprocal(out=deno, in_=dcol)
                        wvt = wvp.tile([TS, C], BF16, tag=f"wvt{o}", name=f"wvt{o}")
                        nc.vector.tensor_scalar(
                            out=wvt, in0=psP[o][:, 0:128],
                            scalar1=deno, scalar2=rsqall[o][:, i:i + 1],
                            op0=ALU.mult, op1=ALU.mult)
                        pwp = pW.tile([C, TS], BF16, tag="pw", name="pwt")
                        nc.tensor.transpose(pwp, wvt, ident)
                        wvn[o] = wvp.tile([C, TS], BF16, tag=f"wvn{o}", name=f"wvn{o}")
                        nc.scalar.copy(out=wvn[o], in_=pwp)
                    psO = pB.tile([C, TS], F32, tag="pb", name="psO")
                    nc.tensor.matmul(psO, boutrow, onesrow, start=True, stop=False,
                                     skip_group_check=True)
                    nc.tensor.matmul(psO, WVT[0], wvn[0], start=False, stop=False,
                                     skip_group_check=True)
                    nc.tensor.matmul(psO, WVT[1], wvn[1], start=False, stop=True,
                                     skip_group_check=True)
                    osb = osbp.tile([C, TS], F32, tag="osb", name="osb")
                    nc.scalar.copy(out=osb, in_=psO)
                    nc.sync.dma_start(out=out_d[b, :, i * TS:(i + 1) * TS], in_=osb)

    nc.finalize()
    return nc


_CACHE = {}


def _get_nc(scal):
    key = tuple(sorted(scal.items()))
    if key not in _CACHE:
        _CACHE[key] = build_nc(scal)
    return _CACHE[key]


def kernel(**inputs):
    inp = {k: np.asarray(v, dtype=np.float32) for k, v in inputs.items()}
    B = inp["tensor1"].shape[0]
    x1 = inp["tensor1"].reshape(B, C, N)
    x2 = inp["tensor2"].reshape(B, C, N)

    wq1, bq1 = inp["wq1"], inp["bq1"]
    wk1, bk1 = inp["wk1"], inp["bk1"]
    wv1, bv1 = inp["wv1"], inp["bv1"]
    wq2, bq2 = inp["wq2"], inp["bq2"]
    wk2, bk2 = inp["wk2"], inp["bk2"]
    wv2, bv2 = inp["wv2"], inp["bv2"]
    wr1, br1 = inp["wr1"], inp["br1"]
    wr2, br2 = inp["wr2"], inp["br2"]
    wcat, bcat = inp["wcat"], inp["bcat"]

    wcat1, wcat2 = wcat[:, :C], wcat[:, C:]
    WV1 = wcat1 @ wr2
    WV2 = wcat2 @ wr1
    bout = wcat1 @ br2 + wcat2 @ br1 + bcat

    def pack_qkv(wq, bq, wk, bk, wv):
        return np.concatenate(
            [wq.T, wk.T, wv.T, (wq.T @ bq)[:, None], 2.0 * (wk.T @ bk)[:, None]],
            axis=1).astype(BF)

    scal = {
        "bq1sq": float(bq1 @ bq1), "bk1sq": float(bk1 @ bk1),
        "bq2sq": float(bq2 @ bq2), "bk2sq": float(bk2 @ bk2),
    }
    nc = _get_nc(scal)

    consts = {
        "wqkv1": pack_qkv(wq1, bq1, wk1, bk1, wv1),
        "wqkv2": pack_qkv(wq2, bq2, wk2, bk2, wv2),
        "wqT1": np.ascontiguousarray(wq1.T).astype(BF),
        "wqT2": np.ascontiguousarray(wq2.T).astype(BF),
        "WV1T": np.ascontiguousarray(WV1.T).astype(BF),
        "WV2T": np.ascontiguousarray(WV2.T).astype(BF),
        "ident": np.eye(C, dtype=np.float32).astype(BF),
        "onesrow": np.ones((1, C), np.float32).astype(BF),
        "boutrow": bout[None, :].astype(BF),
        "bqb": np.stack([bq1, bq2], axis=1).astype(BF),
        "bcolsf": np.stack([bq1, bk1, bq2, bk2], axis=1).astype(np.float32),
        "bvN": np.concatenate([N * bv1, N * bv2])[None, :].astype(np.float32),
        "bkrows": np.stack([bk1, bk2], axis=0).astype(BF),
        "bvrows": np.stack([bv1, bv2], axis=0).astype(BF),
    }

    in_maps = []
    for cid in range(NCORES):
        m = dict(consts)
        m["x1"] = np.ascontiguousarray(x1[cid * BPC:(cid + 1) * BPC])
        m["x2"] = np.ascontiguousarray(x2[cid * BPC:(cid + 1) * BPC])
        in_maps.append(m)

    import kernel as _self
    res = run_bass_kernel_spmd(nc, in_maps, core_ids=list(range(NCORES)),
                               trace=getattr(_self, "TRACE", False))
    _self.LAST_RESULT = res
    out = np.concatenate([r["out"] for r in res.results], axis=0)
    return out.reshape(B, C, 128, 128).astype(np.float32)


TRACE = False
LAST_RESULT = None



# revision 4
# speedup vs baseline: 1.3198x; 1.3198x over previous
"""Trainium2 Bass kernel for nn_CrossAttention (B=16, C=128, N=128*128).

Sharding: data-parallel over batch across 8 cores (2 batches/core).

Per-batch decomposition (validated against the reference):
  For side s, x in bf16 (host-cast). qnat_b = wq x + bq (natural layout,
  bias folded into the PSUM->SBUF copy). nq[n] = sum_c qnat_b^2 via a
  ones-column matmul over qsq = qnat_b*qnat_b, accumulated as rows and
  DMA-reshaped to [128, NT] stat tiles for batched sqrt/recip.
  psA[n, 0:128]=k_raw, [128:256]=v_raw, [256]=x^T(2 wk^T bk) per 128-tile;
  k-norm: sumsq_k = ACTsq(k_raw) + crossk + |bk|^2, rsk = rsqrt.
  matcomb[m,0:129] += (k_raw*rsk)^T [v_raw|1]; corr rows via cl=[rsk|1].
  mat_full = mat_mm + outer(e-EPS,bv) + outer(bk,B), e = A + bk*sigma + EPS.
  Pass 2 per tile: P_s = sq_s x vsum_s + qnat_b_s^T mat_s (bias term folds
  in because qnat carries bq); P_s[:,128] = qnat_b_s^T e_other so
  den = 1/(N + rsq_s * P_s[:,128]) directly. wv^T = P[:,0:128]*den*rsq,
  transposed per tile into a [C,512] chunk, then one 512-wide fused
  output conv (bout + WV0 wv0 + WV1 wv1) and one bf16 store per chunk.
"""

import numpy as np
import ml_dtypes

import concourse.bass as bass
from concourse import bacc
import concourse.mybir as mybir
import concourse.tile as tile
from concourse.bass_utils import run_bass_kernel_spmd

BPC = 2          # batches per core
NCORES = 8
C = 128
N = 128 * 128
TS = 128         # n-tile size
NT = N // TS     # 128 tiles
CH = 512         # chunk width
NCH = N // CH    # 32 chunks
TPC = CH // TS   # tiles per chunk = 4
EPS = 1e-6

F32 = mybir.dt.float32
BF16 = mybir.dt.bfloat16
AF = mybir.ActivationFunctionType
ALU = mybir.AluOpType
BF = ml_dtypes.bfloat16


def build_nc(scal):
    nc = bacc.Bacc("TRN2", target_bir_lowering=False)

    x1_d = nc.dram_tensor("x1", [BPC, C, N], BF16, kind="ExternalInput")
    x2_d = nc.dram_tensor("x2", [BPC, C, N], BF16, kind="ExternalInput")
    out_d = nc.dram_tensor("out", [BPC, C, N], BF16, kind="ExternalOutput")
    wkv1_d = nc.dram_tensor("wkv1", [C, 257], BF16, kind="ExternalInput")
    wkv2_d = nc.dram_tensor("wkv2", [C, 257], BF16, kind="ExternalInput")
    wqT1_d = nc.dram_tensor("wqT1", [C, C], BF16, kind="ExternalInput")
    wqT2_d = nc.dram_tensor("wqT2", [C, C], BF16, kind="ExternalInput")
    WV1T_d = nc.dram_tensor("WV1T", [C, C], BF16, kind="ExternalInput")
    WV2T_d = nc.dram_tensor("WV2T", [C, C], BF16, kind="ExternalInput")
    ident_d = nc.dram_tensor("ident", [C, C], BF16, kind="ExternalInput")
    onescol_d = nc.dram_tensor("onescol", [C, 1], BF16, kind="ExternalInput")
    ones512_d = nc.dram_tensor("ones512", [1, CH], BF16, kind="ExternalInput")
    boutrow_d = nc.dram_tensor("boutrow", [1, C], BF16, kind="ExternalInput")
    bcolsf_d = nc.dram_tensor("bcolsf", [C, 4], F32, kind="ExternalInput")
    bvN_d = nc.dram_tensor("bvN", [1, 2 * C], F32, kind="ExternalInput")
    bkrows_d = nc.dram_tensor("bkrows", [2, C], BF16, kind="ExternalInput")
    bvrows_d = nc.dram_tensor("bvrows", [2, C], BF16, kind="ExternalInput")

    with tile.TileContext(nc) as tc:
        with (
            tc.tile_pool(name="consts", bufs=1) as consts,
            tc.tile_pool(name="xch", bufs=3) as xch,
            tc.tile_pool(name="qsqp", bufs=2) as qsqp,
            tc.tile_pool(name="kv", bufs=3) as kv,
            tc.tile_pool(name="qnat", bufs=1) as qnatp,
            tc.tile_pool(name="rowp", bufs=1) as rowp,
            tc.tile_pool(name="stats", bufs=1) as stats,
            tc.tile_pool(name="cols", bufs=6) as colsp,
            tc.tile_pool(name="wvch", bufs=2) as wvchp,
            tc.tile_pool(name="osb", bufs=3) as osbp,
            tc.tile_pool(name="junk", bufs=2) as junkp,
            tc.tile_pool(name="pA", bufs=2, space="PSUM") as pA,
            tc.tile_pool(name="pB", bufs=2, space="PSUM") as pB,
            tc.tile_pool(name="pW", bufs=2, space="PSUM") as pW,
            tc.tile_pool(name="pM", bufs=1, space="PSUM") as pM,
        ):
            # ---- constants ----
            wkv = [consts.tile([C, 257], BF16, tag=f"wkv{s}", name=f"wkv{s}")
                   for s in range(2)]
            nc.sync.dma_start(out=wkv[0], in_=wkv1_d[:, :])
            nc.sync.dma_start(out=wkv[1], in_=wkv2_d[:, :])
            wqT = [consts.tile([C, C], BF16, tag=f"wqT{s}", name=f"wqT{s}")
                   for s in range(2)]
            nc.sync.dma_start(out=wqT[0], in_=wqT1_d[:, :])
            nc.sync.dma_start(out=wqT[1], in_=wqT2_d[:, :])
            WVT = [consts.tile([C, C], BF16, tag=f"WVT{s}", name=f"WVT{s}")
                   for s in range(2)]
            nc.sync.dma_start(out=WVT[0], in_=WV1T_d[:, :])
            nc.sync.dma_start(out=WVT[1], in_=WV2T_d[:, :])
            ident = consts.tile([C, C], BF16, tag="ident")
            nc.sync.dma_start(out=ident, in_=ident_d[:, :])
            onescol = consts.tile([C, 1], BF16, tag="onescol")
            nc.sync.dma_start(out=onescol, in_=onescol_d[:, :])
            ones512 = consts.tile([1, CH], BF16, tag="ones512")
            nc.sync.dma_start(out=ones512, in_=ones512_d[:, :])
            boutrow = consts.tile([1, C], BF16, tag="boutrow")
            nc.sync.dma_start(out=boutrow, in_=boutrow_d[:, :])
            bcolsf = consts.tile([C, 4], F32, tag="bcolsf")
            nc.sync.dma_start(out=bcolsf, in_=bcolsf_d[:, :])
            bvN = consts.tile([1, 2 * C], F32, tag="bvN")
            nc.sync.dma_start(out=bvN, in_=bvN_d[:, :])
            eM = [consts.tile([2, C], BF16, tag=f"eM{s}", name=f"eM{s}")
                  for s in range(2)]
            nc.sync.dma_start(out=eM[0][1:2, :], in_=bkrows_d[0:1, :])
            nc.sync.dma_start(out=eM[1][1:2, :], in_=bkrows_d[1:2, :])
            corrRHS = [consts.tile([2, C], BF16, tag=f"cR{s}", name=f"cR{s}")
                       for s in range(2)]
            nc.sync.dma_start(out=corrRHS[0][0:1, :], in_=bvrows_d[0:1, :])
            nc.sync.dma_start(out=corrRHS[1][0:1, :], in_=bvrows_d[1:2, :])

            xs_d = [x1_d, x2_d]
            betas_k = [scal["bk1sq"], scal["bk2sq"]]

            for b in range(BPC):
                # ---------------- PASS 1 ----------------
                matcomb = pM.tile([C, 512], F32, tag="matcomb")
                corrcomb = pM.tile([4, 512], F32, tag="corrcomb")
                qnat = [qnatp.tile([C, N], BF16, tag=f"qnat{s}", name=f"qnat{s}")
                        for s in range(2)]
                # nq rows during pass 1; reused as sq rows for pass 2
                rowbuf = [rowp.tile([1, N], BF16, tag=f"row{s}", name=f"row{s}")
                          for s in range(2)]
                nkall = [stats.tile([TS, NT], F32, tag=f"nk{s}", name=f"nk{s}")
                         for s in range(2)]

                for ch in range(NCH):
                    xb = [xch.tile([C, CH], BF16, tag=f"xb{s}", name=f"xb{s}")
                          for s in range(2)]
                    for s in range(2):
                        nc.sync.dma_start(
                            out=xb[s], in_=xs_d[s][b, :, ch * CH:(ch + 1) * CH])
                    for s in range(2):
                        pq = pB.tile([C, CH], F32, tag="pb", name="pq")
                        nc.tensor.matmul(pq, wqT[s], xb[s], start=True, stop=True)
                        qch = qnat[s][:, ch * CH:(ch + 1) * CH]
                        nc.scalar.activation(
                            out=qch, in_=pq, func=AF.Identity,
                            bias=bcolsf[:, 2 * s:2 * s + 1], scale=1.0)
                        qsq = qsqp.tile([C, CH], BF16, tag="qsq", name="qsq")
                        nc.vector.tensor_tensor(
                            out=qsq, in0=qch, in1=qch, op=ALU.mult)
                        pnq = pB.tile([1, CH], F32, tag="pb", name="pnq")
                        nc.tensor.matmul(pnq, onescol, qsq, start=True, stop=True)
                        nc.scalar.copy(
                            out=rowbuf[s][0:1, ch * CH:(ch + 1) * CH], in_=pnq)
                    for t in range(TPC):
                        i = ch * TPC + t
                        for s in range(2):
                            moff = 256 * s
                            psA = pA.tile([TS, 257], F32, tag="ps", name="psA")
                            nc.tensor.matmul(
                                psA, xb[s][:, t * TS:(t + 1) * TS], wkv[s],
                                start=True, stop=True)
                            ja = junkp.tile([TS, C], BF16, tag="junkA", name="ja")
                            nkcol = colsp.tile([TS, 1], F32, tag="nk", name="nkcol")
                            nc.scalar.activation(
                                out=ja, in_=psA[:, 0:128], func=AF.Square,
                                accum_out=nkcol)
                            tcol = colsp.tile([TS, 1], F32, tag="tk", name="tcol")
                            nc.vector.tensor_scalar(
                                out=tcol, in0=psA[:, 256:257],
                                scalar1=float(betas_k[s]), scalar2=None,
                                op0=ALU.add)
                            skcol = colsp.tile([TS, 1], F32, tag="sk", name="skcol")
                            nc.scalar.activation(
                                out=skcol, in_=nkcol, func=AF.Sqrt,
                                bias=tcol, scale=1.0)
                            rskcol = colsp.tile([TS, 1], F32, tag="rsk",
                                                name="rskcol")
                            nc.vector.reciprocal(out=rskcol, in_=skcol)
                            cl = kv.tile([TS, 2], BF16, tag=f"cl{s}", name=f"cl{s}")
                            nc.vector.tensor_copy(out=cl[:, 0:1], in_=rskcol)
                            nc.vector.memset(cl[:, 1:2], 1.0)
                            ks = kv.tile([TS, C], BF16, tag=f"k{s}", name=f"ks{s}")
                            nc.vector.tensor_scalar(
                                out=ks, in0=psA[:, 0:128],
                                scalar1=rskcol, scalar2=None, op0=ALU.mult)
                            vs = kv.tile([TS, C + 4], BF16, tag=f"v{s}",
                                         name=f"vs{s}")
                            nc.scalar.copy(out=vs[:, 0:128], in_=psA[:, 128:256])
                            nc.vector.memset(vs[:, 128:129], 1.0)
                            nc.tensor.matmul(
                                matcomb[:, moff:moff + 129], ks, vs[:, 0:129],
                                start=(i == 0), stop=False, skip_group_check=True)
                            nc.tensor.matmul(
                                corrcomb[0:2, moff:moff + 129], cl, vs[:, 0:129],
                                start=(i == 0), stop=(i == NT - 1),
                                skip_group_check=True)

                # ---------------- PASS 1.5 ----------------
                rsqall = [stats.tile([TS, NT], F32, tag=f"rsq{s}", name=f"rsq{s}")
                          for s in range(2)]
                matR = [stats.tile([C, 132], BF16, tag=f"matR{s}", name=f"matR{s}")
                        for s in range(2)]
                vsumrowX = [stats.tile([1, 132], BF16, tag=f"vsX{s}",
                                       name=f"vsX{s}") for s in range(2)]
                ecolf = [None, None]
                ecolb = [None, None]
                for s in range(2):
                    moff = 256 * s
                    # q norms: row -> [tile, pos] via simple partition scatter,
                    # sq in that layout (row-flattens back), rsq via transpose
                    nqTsw = stats.tile([NT, TS], BF16, tag=f"nqT{s}",
                                       name=f"nqTsw{s}")
                    nc.sync.dma_start(out=nqTsw, in_=rowbuf[s][0:1, :])
                    sqsw = stats.tile([NT, TS], BF16, tag=f"sqT{s}",
                                      name=f"sqsw{s}")
                    nc.scalar.activation(out=sqsw, in_=nqTsw, func=AF.Sqrt)
                    nc.sync.dma_start(out=rowbuf[s], in_=sqsw)  # now the sq row
                    pt = pW.tile([TS, NT], BF16, tag="pw", name="ptr")
                    nc.tensor.transpose(pt, sqsw, ident)
                    nc.vector.reciprocal(out=rsqall[s], in_=pt)
                    # e col and mat fixups
                    sig1 = stats.tile([1, 1], F32, tag=f"sg{s}", name=f"sg{s}")
                    nc.scalar.copy(out=sig1,
                                   in_=corrcomb[0:1, moff + 128:moff + 129])
                    sigc = stats.tile([C, 1], F32, tag=f"sgc{s}", name=f"sgc{s}")
                    nc.gpsimd.partition_broadcast(sigc, sig1)
                    e0 = stats.tile([C, 1], F32, tag=f"e0{s}", name=f"e0{s}")
                    nc.vector.tensor_scalar(
                        out=e0, in0=bcolsf[:, 2 * s + 1:2 * s + 2],
                        scalar1=sigc, scalar2=None, op0=ALU.mult)
                    nc.vector.tensor_tensor(
                        out=e0, in0=e0, in1=matcomb[:, moff + 128:moff + 129],
                        op=ALU.add)
                    ecol = stats.tile([C, 1], F32, tag=f"ec{s}", name=f"ec{s}")
                    nc.vector.tensor_scalar(
                        out=ecol, in0=e0, scalar1=EPS, scalar2=None, op0=ALU.add)
                    ecolf[s] = ecol
                    eb = stats.tile([C, 1], BF16, tag=f"ecb{s}", name=f"ecb{s}")
                    nc.vector.tensor_copy(out=eb, in_=ecol)
                    ecolb[s] = eb
                    e0b = stats.tile([C, 1], BF16, tag=f"e0b{s}", name=f"e0b{s}")
                    nc.vector.tensor_copy(out=e0b, in_=e0)
                    pe = pW.tile([1, C], BF16, tag="pw", name="per")
                    nc.tensor.transpose(pe, e0b, ident)
                    nc.scalar.copy(out=eM[s][0:1, :], in_=pe)
                    Bb = stats.tile([1, C], BF16, tag=f"Bb{s}", name=f"Bb{s}")
                    nc.vector.tensor_copy(out=Bb,
                                          in_=corrcomb[0:1, moff:moff + 128])
                    nc.sync.dma_start(out=corrRHS[s][1:2, :], in_=Bb)
                    nc.tensor.matmul(
                        matcomb[:, moff:moff + 128], eM[s], corrRHS[s],
                        start=False, stop=True, skip_group_check=True)
                    nc.vector.tensor_copy(out=matR[s][:, 0:128],
                                          in_=matcomb[:, moff:moff + 128])
                    corrsb = stats.tile([2, C], F32, tag=f"csb{s}", name=f"csb{s}")
                    nc.scalar.copy(out=corrsb, in_=corrcomb[0:2, moff:moff + 128])
                    vsr = stats.tile([1, C], F32, tag=f"vsr{s}", name=f"vsr{s}")
                    nc.sync.dma_start(out=vsr, in_=corrsb[1:2, :])
                    vsf = stats.tile([1, C], F32, tag=f"vsf{s}", name=f"vsf{s}")
                    nc.vector.tensor_tensor(
                        out=vsf, in0=vsr, in1=bvN[0:1, s * C:(s + 1) * C],
                        op=ALU.add)
                    nc.vector.memset(vsumrowX[s][:, 128:132], 0.0)
                    nc.vector.tensor_copy(out=vsumrowX[s][:, 0:128], in_=vsf)
                for s in range(2):
                    nc.vector.tensor_copy(out=matR[s][:, 128:129],
                                          in_=ecolb[1 - s])

                # ---------------- PASS 2 ----------------
                for ch in range(NCH):
                    wvchunk = [wvchp.tile([C, CH], BF16, tag=f"wvc{s}",
                                          name=f"wvc{s}") for s in range(2)]
                    for t in range(TPC):
                        i = ch * TPC + t
                        psP = [None, None]
                        for s in range(2):
                            psP[s] = pA.tile([TS, 257], F32, tag="ps", name="psP")
                            nc.tensor.matmul(
                                psP[s][:, 0:129],
                                rowbuf[s][0:1, i * TS:(i + 1) * TS],
                                vsumrowX[s][:, 0:129],
                                start=True, stop=False, skip_group_check=True)
                            nc.tensor.matmul(
                                psP[s][:, 0:129], qnat[s][:, i * TS:(i + 1) * TS],
                                matR[s][:, 0:129],
                                start=False, stop=True, skip_group_check=True)
                        for s in range(2):
                            o = 1 - s
                            dcol = colsp.tile([TS, 1], F32, tag="dt", name="dcol")
                            nc.vector.tensor_scalar(
                                out=dcol, in0=psP[s][:, 128:129],
                                scalar1=rsqall[s][:, i:i + 1],
                                scalar2=float(N), op0=ALU.mult, op1=ALU.add)
                            deno = colsp.tile([TS, 1], F32, tag="den", name="deno")
                            nc.vector.reciprocal(out=deno, in_=dcol)
                            wvt = junkp.tile([TS, C], BF16, tag=f"wvt{o}",
                                             name=f"wvt{o}")
                            nc.vector.tensor_scalar(
                                out=wvt, in0=psP[o][:, 0:128],
                                scalar1=deno, scalar2=rsqall[o][:, i:i + 1],
                                op0=ALU.mult, op1=ALU.mult)
                            pwp = pW.tile([C, TS], BF16, tag="pw", name="pwt")
                            nc.tensor.transpose(pwp, wvt, ident)
                            nc.scalar.copy(
                                out=wvchunk[o][:, t * TS:(t + 1) * TS], in_=pwp)
                    psO = pB.tile([C, CH], F32, tag="pb", name="psO")
                    nc.tensor.matmul(psO, boutrow, ones512, start=True,
                                     stop=False, skip_group_check=True)
                    nc.tensor.matmul(psO, WVT[0], wvchunk[0], start=False,
                                     stop=False, skip_group_check=True)
                    nc.tensor.matmul(psO, WVT[1], wvchunk[1], start=False,
                                     stop=True, skip_group_check=True)
                    osb = osbp.tile([C, CH], BF16, tag="osb", name="osb")
                    nc.scalar.copy(out=osb, in_=psO)
                    nc.sync.dma_start(
                        out=out_d[b, :, ch * CH:(ch + 1) * CH], in_=osb)

    nc.finalize()
    return nc


_CACHE = {}


def _get_nc(scal):
    key = tuple(sorted(scal.items()))
    if key not in _CACHE:
        _CACHE[key] = build_nc(scal)
    return _CACHE[key]


def kernel(**inputs):
    inp = {k: np.asarray(v, dtype=np.float32) for k, v in inputs.items()}
    B = inp["tensor1"].shape[0]
    x1 = inp["tensor1"].reshape(B, C, N).astype(BF)
    x2 = inp["tensor2"].reshape(B, C, N).astype(BF)

    wq1, bq1 = inp["wq1"], inp["bq1"]
    wk1, bk1 = inp["wk1"], inp["bk1"]
    wv1 = inp["wv1"]
    wq2, bq2 = inp["wq2"], inp["bq2"]
    wk2, bk2 = inp["wk2"], inp["bk2"]
    wv2 = inp["wv2"]
    bv1, bv2 = inp["bv1"], inp["bv2"]
    wr1, br1 = inp["wr1"], inp["br1"]
    wr2, br2 = inp["wr2"], inp["br2"]
    wcat, bcat = inp["wcat"], inp["bcat"]

    wcat1, wcat2 = wcat[:, :C], wcat[:, C:]
    WV1 = wcat1 @ wr2
    WV2 = wcat2 @ wr1
    bout = wcat1 @ br2 + wcat2 @ br1 + bcat

    def pack_kv(wk, bk, wv):
        return np.concatenate(
            [wk.T, wv.T, 2.0 * (wk.T @ bk)[:, None]], axis=1).astype(BF)

    scal = {"bk1sq": float(bk1 @ bk1), "bk2sq": float(bk2 @ bk2)}
    nc = _get_nc(scal)

    consts = {
        "wkv1": pack_kv(wk1, bk1, wv1),
        "wkv2": pack_kv(wk2, bk2, wv2),
        "wqT1": np.ascontiguousarray(wq1.T).astype(BF),
        "wqT2": np.ascontiguousarray(wq2.T).astype(BF),
        "WV1T": np.ascontiguousarray(WV1.T).astype(BF),
        "WV2T": np.ascontiguousarray(WV2.T).astype(BF),
        "ident": np.eye(C, dtype=np.float32).astype(BF),
        "onescol": np.ones((C, 1), np.float32).astype(BF),
        "ones512": np.ones((1, CH), np.float32).astype(BF),
        "boutrow": bout[None, :].astype(BF),
        "bcolsf": np.stack([bq1, bk1, bq2, bk2], axis=1).astype(np.float32),
        "bvN": np.concatenate([N * bv1, N * bv2])[None, :].astype(np.float32),
        "bkrows": np.stack([bk1, bk2], axis=0).astype(BF),
        "bvrows": np.stack([bv1, bv2], axis=0).astype(BF),
    }

    in_maps = []
    for cid in range(NCORES):
        m = dict(consts)
        m["x1"] = np.ascontiguousarray(x1[cid * BPC:(cid + 1) * BPC])
        m["x2"] = np.ascontiguousarray(x2[cid * BPC:(cid + 1) * BPC])
        in_maps.append(m)

    import kernel as _self
    res = run_bass_kernel_spmd(nc, in_maps, core_ids=list(range(NCORES)),
                               trace=getattr(_self, "TRACE", False))
    _self.LAST_RESULT = res
    out = np.concatenate([np.asarray(r["out"]) for r in res.results], axis=0)
    return out.reshape(B, C, 128, 128).astype(np.float32)


TRACE = False
LAST_RESULT = None


# revision 21
# speedup vs baseline: 1.3959x; 1.0576x over previous
"""Trainium2 Bass kernel for nn_CrossAttention (B=16, C=128, N=128*128).

Sharding: data-parallel over batch across 8 cores (2 batches/core).

Per-batch decomposition (validated against the reference):
  For side s, x in bf16 (host-cast). qnat_b = wq x + bq (natural layout,
  bias folded into the PSUM->SBUF copy). nq[n] = sum_c qnat_b^2 via a
  ones-column matmul over qsq = qnat_b*qnat_b, accumulated as rows and
  DMA-reshaped to [128, NT] stat tiles for batched sqrt/recip.
  psA[n, 0:128]=k_raw, [128:256]=v_raw, [256]=x^T(2 wk^T bk) per 128-tile;
  k-norm: sumsq_k = ACTsq(k_raw) + crossk + |bk|^2, rsk = rsqrt.
  matcomb[m,0:129] += (k_raw*rsk)^T [v_raw|1]; corr rows via cl=[rsk|1].
  mat_full = mat_mm + outer(e-EPS,bv) + outer(bk,B), e = A + bk*sigma + EPS.
  Pass 2 per tile: P_s = sq_s x vsum_s + qnat_b_s^T mat_s (bias term folds
  in because qnat carries bq); P_s[:,128] = qnat_b_s^T e_other so
  den = 1/(N + rsq_s * P_s[:,128]) directly. wv^T = P[:,0:128]*den*rsq,
  transposed per tile into a [C,512] chunk, then one 512-wide fused
  output conv (bout + WV0 wv0 + WV1 wv1) and one bf16 store per chunk.
"""

import numpy as np
import ml_dtypes

import concourse.bass as bass
from concourse import bacc
import concourse.mybir as mybir
import concourse.tile as tile
from concourse.bass_utils import run_bass_kernel_spmd

BPC = 2          # batches per core
NCORES = 8
C = 128
N = 128 * 128
TS = 128         # n-tile size
NT = N // TS     # 128 tiles
CH = 512         # chunk width
NCH = N // CH    # 32 chunks
TPC = CH // TS   # tiles per chunk = 4
EPS = 1e-6

F32 = mybir.dt.float32
BF16 = mybir.dt.bfloat16
AF = mybir.ActivationFunctionType
ALU = mybir.AluOpType
BF = ml_dtypes.bfloat16


def build_nc(scal):
    nc = bacc.Bacc("TRN2", target_bir_lowering=False)

    x1_d = nc.dram_tensor("x1", [BPC, C, N], BF16, kind="ExternalInput")
    x2_d = nc.dram_tensor("x2", [BPC, C, N], BF16, kind="ExternalInput")
    out_d = nc.dram_tensor("out", [BPC, C, N], BF16, kind="ExternalOutput")
    wkv1_d = nc.dram_tensor("wkv1", [C, 386], BF16, kind="ExternalInput")
    wkv2_d = nc.dram_tensor("wkv2", [C, 386], BF16, kind="ExternalInput")
    wqT1_d = nc.dram_tensor("wqT1", [C, C], BF16, kind="ExternalInput")
    wqT2_d = nc.dram_tensor("wqT2", [C, C], BF16, kind="ExternalInput")
    WV1T_d = nc.dram_tensor("WV1T", [C, C], BF16, kind="ExternalInput")
    WV2T_d = nc.dram_tensor("WV2T", [C, C], BF16, kind="ExternalInput")
    ident_d = nc.dram_tensor("ident", [C, C], BF16, kind="ExternalInput")
    ones512_d = nc.dram_tensor("ones512", [1, CH], BF16, kind="ExternalInput")
    boutrow_d = nc.dram_tensor("boutrow", [1, C], BF16, kind="ExternalInput")
    bcolsf_d = nc.dram_tensor("bcolsf", [C, 4], F32, kind="ExternalInput")
    betas_d = nc.dram_tensor("betas", [C, 4], F32, kind="ExternalInput")
    bvN_d = nc.dram_tensor("bvN", [1, 2 * C], F32, kind="ExternalInput")
    bkrows_d = nc.dram_tensor("bkrows", [2, C], BF16, kind="ExternalInput")
    bvrows_d = nc.dram_tensor("bvrows", [2, C], BF16, kind="ExternalInput")

    with tile.TileContext(nc) as tc:
        with (
            tc.tile_pool(name="consts", bufs=1) as consts,
            tc.tile_pool(name="xch", bufs=2) as xch,
            tc.tile_pool(name="kvq", bufs=1) as kvqp,
            tc.tile_pool(name="kv", bufs=3) as kv,
            tc.tile_pool(name="qnat", bufs=1) as qnatp,
            tc.tile_pool(name="rowp", bufs=1) as rowp,
            tc.tile_pool(name="stats", bufs=1) as stats,
            tc.tile_pool(name="cols", bufs=6) as colsp,
            tc.tile_pool(name="wvch", bufs=2) as wvchp,
            tc.tile_pool(name="osb", bufs=3) as osbp,
            tc.tile_pool(name="junk", bufs=2) as junkp,
            tc.tile_pool(name="pA", bufs=2, space="PSUM") as pA,
            tc.tile_pool(name="pB", bufs=2, space="PSUM") as pB,
            tc.tile_pool(name="pW", bufs=2, space="PSUM") as pW,
            tc.tile_pool(name="pM", bufs=1, space="PSUM") as pM,
        ):
            # ---- constants ----
            wkv = [consts.tile([C, 386], BF16, tag=f"wkv{s}", name=f"wkv{s}")
                   for s in range(2)]
            nc.sync.dma_start(out=wkv[0], in_=wkv1_d[:, :])
            nc.sync.dma_start(out=wkv[1], in_=wkv2_d[:, :])
            wqT = [consts.tile([C, C], BF16, tag=f"wqT{s}", name=f"wqT{s}")
                   for s in range(2)]
            nc.sync.dma_start(out=wqT[0], in_=wqT1_d[:, :])
            nc.sync.dma_start(out=wqT[1], in_=wqT2_d[:, :])
            WVT = [consts.tile([C, C], BF16, tag=f"WVT{s}", name=f"WVT{s}")
                   for s in range(2)]
            nc.sync.dma_start(out=WVT[0], in_=WV1T_d[:, :])
            nc.sync.dma_start(out=WVT[1], in_=WV2T_d[:, :])
            ident = consts.tile([C, C], BF16, tag="ident")
            nc.sync.dma_start(out=ident, in_=ident_d[:, :])
            ones512 = consts.tile([1, CH], BF16, tag="ones512")
            nc.sync.dma_start(out=ones512, in_=ones512_d[:, :])
            boutrow = consts.tile([1, C], BF16, tag="boutrow")
            nc.sync.dma_start(out=boutrow, in_=boutrow_d[:, :])
            bcolsf = consts.tile([C, 4], F32, tag="bcolsf")
            nc.sync.dma_start(out=bcolsf, in_=bcolsf_d[:, :])
            betasc = consts.tile([C, 4], F32, tag="betasc")
            nc.sync.dma_start(out=betasc, in_=betas_d[:, :])
            bvN = consts.tile([1, 2 * C], F32, tag="bvN")
            nc.sync.dma_start(out=bvN, in_=bvN_d[:, :])
            eM = [consts.tile([2, C], BF16, tag=f"eM{s}", name=f"eM{s}")
                  for s in range(2)]
            nc.sync.dma_start(out=eM[0][1:2, :], in_=bkrows_d[0:1, :])
            nc.sync.dma_start(out=eM[1][1:2, :], in_=bkrows_d[1:2, :])
            corrRHS = [consts.tile([2, C], BF16, tag=f"cR{s}", name=f"cR{s}")
                       for s in range(2)]
            nc.sync.dma_start(out=corrRHS[0][0:1, :], in_=bvrows_d[0:1, :])
            nc.sync.dma_start(out=corrRHS[1][0:1, :], in_=bvrows_d[1:2, :])

            xs_d = [x1_d, x2_d]
            betas_q = [scal["bq1sq"], scal["bq2sq"]]
            betas_k = [scal["bk1sq"], scal["bk2sq"]]
            QC = NCH // 4            # 8 chunks per quarter
            QTT = QC * TPC           # 32 tiles per quarter

            for b in range(BPC):
                # ---------------- PASS 1 ----------------
                matcomb = pM.tile([C, 512], F32, tag="matcomb")
                corrcomb = pM.tile([4, 512], F32, tag="corrcomb")
                qnat = [qnatp.tile([C, N], BF16, tag=f"qnat{s}", name=f"qnat{s}")
                        for s in range(2)]
                rowbuf = [rowp.tile([1, N], BF16, tag=f"row{s}", name=f"row{s}")
                          for s in range(2)]
                sqall = [stats.tile([TS, NT], F32, tag=f"sqa{s}", name=f"sqa{s}")
                         for s in range(2)]
                rsqall = [stats.tile([TS, NT], F32, tag=f"rsq{s}", name=f"rsq{s}")
                          for s in range(2)]
                # quarter-resident transposed q|k|v|cq|ck tiles (bf16)
                kvq = [kvqp.tile([TS, QTT, 386], BF16, tag=f"kvq{s}",
                                 name=f"kvq{s}") for s in range(2)]
                rskbf = [stats.tile([TS, QTT, 2], BF16, tag=f"rkb{s}",
                                    name=f"rkb{s}") for s in range(2)]
                nqq = [stats.tile([TS, QTT], F32, tag=f"nqq{s}", name=f"nqq{s}")
                       for s in range(2)]
                nkq = [stats.tile([TS, QTT], F32, tag=f"nkq{s}", name=f"nkq{s}")
                       for s in range(2)]

                for q4 in range(4):
                    for c8 in range(QC):
                        ch = q4 * QC + c8
                        xb = [xch.tile([C, CH], BF16, tag=f"xb{s}", name=f"xb{s}")
                              for s in range(2)]
                        for s in range(2):
                            nc.sync.dma_start(
                                out=xb[s],
                                in_=xs_d[s][b, :, ch * CH:(ch + 1) * CH])
                        for s in range(2):
                            pq = pB.tile([C, CH], F32, tag="pb", name="pq")
                            nc.tensor.matmul(pq, wqT[s], xb[s], start=True,
                                             stop=True)
                            qch = qnat[s][:, ch * CH:(ch + 1) * CH]
                            nc.scalar.activation(
                                out=qch, in_=pq, func=AF.Identity,
                                bias=bcolsf[:, 2 * s:2 * s + 1], scale=1.0)
                        for t in range(TPC):
                            t8 = c8 * TPC + t
                            for s in range(2):
                                psA = pA.tile([TS, 386], F32, tag="ps",
                                              name="psA")
                                nc.tensor.matmul(
                                    psA, xb[s][:, t * TS:(t + 1) * TS], wkv[s],
                                    start=True, stop=True)
                                kt = kvq[s][:, t8, :]
                                nc.scalar.copy(out=kt, in_=psA)
                                ja = junkp.tile([TS, C], BF16, tag="junkA",
                                                name="ja")
                                nc.vector.scalar_tensor_tensor(
                                    out=ja, in0=kt[:, 0:128], scalar=1.0,
                                    in1=kt[:, 0:128], op0=ALU.bypass,
                                    op1=ALU.mult,
                                    accum_out=nqq[s][:, t8:t8 + 1])
                                jb = junkp.tile([TS, C], BF16, tag="junkB",
                                                name="jb")
                                nc.vector.scalar_tensor_tensor(
                                    out=jb, in0=kt[:, 128:256], scalar=1.0,
                                    in1=kt[:, 128:256], op0=ALU.bypass,
                                    op1=ALU.mult,
                                    accum_out=nkq[s][:, t8:t8 + 1])
                    # batched norm math for this quarter
                    rskq = [None, None]
                    qsl = slice(q4 * QTT, (q4 + 1) * QTT)
                    for s in range(2):
                        # fold in the (pre-doubled) bias cross terms, then
                        # free the crossq col as the ones col for [v|1]
                        nc.vector.tensor_tensor(
                            out=nqq[s].unsqueeze(2), in0=nqq[s].unsqueeze(2),
                            in1=kvq[s][:, :, 384:385], op=ALU.add)
                        nc.vector.tensor_tensor(
                            out=nkq[s].unsqueeze(2), in0=nkq[s].unsqueeze(2),
                            in1=kvq[s][:, :, 385:386], op=ALU.add)
                        nc.vector.memset(kvq[s][:, :, 384:385], 1.0)
                        nc.scalar.activation(
                            out=sqall[s][:, qsl], in_=nqq[s], func=AF.Sqrt,
                            bias=betasc[:, 2 * s:2 * s + 1], scale=1.0)
                        nc.vector.reciprocal(out=rsqall[s][:, qsl],
                                             in_=sqall[s][:, qsl])
                        skq = stats.tile([TS, QTT], F32, tag=f"skq{s}",
                                         name=f"skq{s}")
                        nc.scalar.activation(
                            out=skq, in_=nkq[s], func=AF.Sqrt,
                            bias=betasc[:, 2 * s + 1:2 * s + 2], scale=1.0)
                        rskq[s] = stats.tile([TS, QTT], F32, tag=f"rkq{s}",
                                             name=f"rkq{s}")
                        nc.vector.reciprocal(out=rskq[s], in_=skq)
                        nc.vector.tensor_copy(out=rskbf[s][:, :, 0:1],
                                              in_=rskq[s].unsqueeze(2))
                        nc.vector.memset(rskbf[s][:, :, 1:2], 1.0)
                    # matcomb/corrcomb accumulation for this quarter
                    for t8 in range(QTT):
                        i = q4 * QTT + t8
                        for s in range(2):
                            moff = 256 * s
                            ks = kv.tile([TS, C], BF16, tag=f"k{s}",
                                         name=f"ks{s}")
                            nc.vector.tensor_scalar(
                                out=ks, in0=kvq[s][:, t8, 128:256],
                                scalar1=rskq[s][:, t8:t8 + 1], scalar2=None,
                                op0=ALU.mult)
                            nc.tensor.matmul(
                                matcomb[:, moff:moff + 129], ks,
                                kvq[s][:, t8, 256:385],
                                start=(i == 0), stop=False,
                                skip_group_check=True)
                            nc.tensor.matmul(
                                corrcomb[0:2, moff:moff + 129],
                                rskbf[s][:, t8, :], kvq[s][:, t8, 256:385],
                                start=(i == 0), stop=(i == NT - 1),
                                skip_group_check=True)

                # ---------------- PASS 1.5 ----------------
                matR = [stats.tile([C, 132], BF16, tag=f"matR{s}", name=f"matR{s}")
                        for s in range(2)]
                vsumrowX = [stats.tile([1, 132], BF16, tag=f"vsX{s}",
                                       name=f"vsX{s}") for s in range(2)]
                ecolf = [None, None]
                ecolb = [None, None]
                for s in range(2):
                    moff = 256 * s
                    # sq [pos, tile] -> transpose -> flatten to the sq row
                    sqb = stats.tile([TS, NT], BF16, tag=f"sqb{s}",
                                     name=f"sqb{s}")
                    nc.vector.tensor_copy(out=sqb, in_=sqall[s])
                    pt = pW.tile([TS, NT], BF16, tag="pw", name="ptr")
                    nc.tensor.transpose(pt, sqb, ident)
                    sqTsw = stats.tile([NT, TS], BF16, tag=f"sqT{s}",
                                       name=f"sqTsw{s}")
                    nc.scalar.copy(out=sqTsw, in_=pt)
                    nc.sync.dma_start(out=rowbuf[s], in_=sqTsw)
                    # e col and mat fixups
                    sig1 = stats.tile([1, 1], F32, tag=f"sg{s}", name=f"sg{s}")
                    nc.scalar.copy(out=sig1,
                                   in_=corrcomb[0:1, moff + 128:moff + 129])
                    sigc = stats.tile([C, 1], F32, tag=f"sgc{s}", name=f"sgc{s}")
                    nc.gpsimd.partition_broadcast(sigc, sig1)
                    e0 = stats.tile([C, 1], F32, tag=f"e0{s}", name=f"e0{s}")
                    nc.vector.tensor_scalar(
                        out=e0, in0=bcolsf[:, 2 * s + 1:2 * s + 2],
                        scalar1=sigc, scalar2=None, op0=ALU.mult)
                    nc.vector.tensor_tensor(
                        out=e0, in0=e0, in1=matcomb[:, moff + 128:moff + 129],
                        op=ALU.add)
                    ecol = stats.tile([C, 1], F32, tag=f"ec{s}", name=f"ec{s}")
                    nc.vector.tensor_scalar(
                        out=ecol, in0=e0, scalar1=EPS, scalar2=None, op0=ALU.add)
                    ecolf[s] = ecol
                    eb = stats.tile([C, 1], BF16, tag=f"ecb{s}", name=f"ecb{s}")
                    nc.vector.tensor_copy(out=eb, in_=ecol)
                    ecolb[s] = eb
                    e0b = stats.tile([C, 1], BF16, tag=f"e0b{s}", name=f"e0b{s}")
                    nc.vector.tensor_copy(out=e0b, in_=e0)
                    pe = pW.tile([1, C], BF16, tag="pw", name="per")
                    nc.tensor.transpose(pe, e0b, ident)
                    nc.scalar.copy(out=eM[s][0:1, :], in_=pe)
                    Bb = stats.tile([1, C], BF16, tag=f"Bb{s}", name=f"Bb{s}")
                    nc.vector.tensor_copy(out=Bb,
                                          in_=corrcomb[0:1, moff:moff + 128])
                    nc.sync.dma_start(out=corrRHS[s][1:2, :], in_=Bb)
                    nc.tensor.matmul(
                        matcomb[:, moff:moff + 128], eM[s], corrRHS[s],
                        start=False, stop=True, skip_group_check=True)
                    nc.vector.tensor_copy(out=matR[s][:, 0:128],
                                          in_=matcomb[:, moff:moff + 128])
                    corrsb = stats.tile([2, C], F32, tag=f"csb{s}", name=f"csb{s}")
                    nc.scalar.copy(out=corrsb, in_=corrcomb[0:2, moff:moff + 128])
                    vsr = stats.tile([1, C], F32, tag=f"vsr{s}", name=f"vsr{s}")
                    nc.sync.dma_start(out=vsr, in_=corrsb[1:2, :])
                    vsf = stats.tile([1, C], F32, tag=f"vsf{s}", name=f"vsf{s}")
                    nc.vector.tensor_tensor(
                        out=vsf, in0=vsr, in1=bvN[0:1, s * C:(s + 1) * C],
                        op=ALU.add)
                    nc.vector.memset(vsumrowX[s][:, 128:132], 0.0)
                    nc.vector.tensor_copy(out=vsumrowX[s][:, 0:128], in_=vsf)
                for s in range(2):
                    nc.vector.tensor_copy(out=matR[s][:, 128:129],
                                          in_=ecolb[1 - s])

                # ---------------- PASS 2 ----------------
                for ch in range(NCH):
                    wvchunk = [wvchp.tile([C, CH], BF16, tag=f"wvc{s}",
                                          name=f"wvc{s}") for s in range(2)]
                    for t in range(TPC):
                        i = ch * TPC + t
                        psP = [None, None]
                        for s in range(2):
                            psP[s] = pA.tile([TS, 257], F32, tag="ps", name="psP")
                            nc.tensor.matmul(
                                psP[s][:, 0:129],
                                rowbuf[s][0:1, i * TS:(i + 1) * TS],
                                vsumrowX[s][:, 0:129],
                                start=True, stop=False, skip_group_check=True)
                            nc.tensor.matmul(
                                psP[s][:, 0:129], qnat[s][:, i * TS:(i + 1) * TS],
                                matR[s][:, 0:129],
                                start=False, stop=True, skip_group_check=True)
                        for s in range(2):
                            o = 1 - s
                            dcol = colsp.tile([TS, 1], F32, tag="dt", name="dcol")
                            nc.vector.tensor_scalar(
                                out=dcol, in0=psP[s][:, 128:129],
                                scalar1=rsqall[s][:, i:i + 1],
                                scalar2=float(N), op0=ALU.mult, op1=ALU.add)
                            deno = colsp.tile([TS, 1], F32, tag="den", name="deno")
                            nc.vector.reciprocal(out=deno, in_=dcol)
                            wvt = junkp.tile([TS, C], BF16, tag=f"wvt{o}",
                                             name=f"wvt{o}")
                            nc.vector.tensor_scalar(
                                out=wvt, in0=psP[o][:, 0:128],
                                scalar1=deno, scalar2=rsqall[o][:, i:i + 1],
                                op0=ALU.mult, op1=ALU.mult)
                            pwp = pW.tile([C, TS], BF16, tag="pw", name="pwt")
                            nc.tensor.transpose(pwp, wvt, ident)
                            nc.scalar.copy(
                                out=wvchunk[o][:, t * TS:(t + 1) * TS], in_=pwp)
                    psO = pB.tile([C, CH], F32, tag="pb", name="psO")
                    nc.tensor.matmul(psO, boutrow, ones512, start=True,
                                     stop=False, skip_group_check=True)
                    nc.tensor.matmul(psO, WVT[0], wvchunk[0], start=False,
                                     stop=False, skip_group_check=True)
                    nc.tensor.matmul(psO, WVT[1], wvchunk[1], start=False,
                                     stop=True, skip_group_check=True)
                    osb = osbp.tile([C, CH], BF16, tag="osb", name="osb")
                    nc.scalar.copy(out=osb, in_=psO)
                    nc.sync.dma_start(
                        out=out_d[b, :, ch * CH:(ch + 1) * CH], in_=osb)

    nc.finalize()
    return nc


_CACHE = {}


def _get_nc(scal):
    key = tuple(sorted(scal.items()))
    if key not in _CACHE:
        _CACHE[key] = build_nc(scal)
    return _CACHE[key]


def kernel(**inputs):
    inp = {k: np.asarray(v, dtype=np.float32) for k, v in inputs.items()}
    B = inp["tensor1"].shape[0]
    x1 = inp["tensor1"].reshape(B, C, N).astype(BF)
    x2 = inp["tensor2"].reshape(B, C, N).astype(BF)

    wq1, bq1 = inp["wq1"], inp["bq1"]
    wk1, bk1 = inp["wk1"], inp["bk1"]
    wv1 = inp["wv1"]
    wq2, bq2 = inp["wq2"], inp["bq2"]
    wk2, bk2 = inp["wk2"], inp["bk2"]
    wv2 = inp["wv2"]
    bv1, bv2 = inp["bv1"], inp["bv2"]
    wr1, br1 = inp["wr1"], inp["br1"]
    wr2, br2 = inp["wr2"], inp["br2"]
    wcat, bcat = inp["wcat"], inp["bcat"]

    wcat1, wcat2 = wcat[:, :C], wcat[:, C:]
    WV1 = wcat1 @ wr2
    WV2 = wcat2 @ wr1
    bout = wcat1 @ br2 + wcat2 @ br1 + bcat

    def pack_kv(wq, bq, wk, bk, wv):
        return np.concatenate(
            [wq.T, wk.T, wv.T, 2.0 * (wq.T @ bq)[:, None],
             2.0 * (wk.T @ bk)[:, None]], axis=1).astype(BF)

    scal = {
        "bq1sq": float(bq1 @ bq1), "bk1sq": float(bk1 @ bk1),
        "bq2sq": float(bq2 @ bq2), "bk2sq": float(bk2 @ bk2),
    }
    nc = _get_nc(scal)

    consts = {
        "wkv1": pack_kv(wq1, bq1, wk1, bk1, wv1),
        "wkv2": pack_kv(wq2, bq2, wk2, bk2, wv2),
        "wqT1": np.ascontiguousarray(wq1.T).astype(BF),
        "wqT2": np.ascontiguousarray(wq2.T).astype(BF),
        "WV1T": np.ascontiguousarray(WV1.T).astype(BF),
        "WV2T": np.ascontiguousarray(WV2.T).astype(BF),
        "ident": np.eye(C, dtype=np.float32).astype(BF),
        "ones512": np.ones((1, CH), np.float32).astype(BF),
        "boutrow": bout[None, :].astype(BF),
        "bcolsf": np.stack([bq1, bk1, bq2, bk2], axis=1).astype(np.float32),
        "betas": np.tile(np.array([scal["bq1sq"], scal["bk1sq"],
                                   scal["bq2sq"], scal["bk2sq"]],
                                  np.float32)[None, :], (C, 1)),
        "bvN": np.concatenate([N * bv1, N * bv2])[None, :].astype(np.float32),
        "bkrows": np.stack([bk1, bk2], axis=0).astype(BF),
        "bvrows": np.stack([bv1, bv2], axis=0).astype(BF),
    }

    in_maps = []
    for cid in range(NCORES):
        m = dict(consts)
        m["x1"] = np.ascontiguousarray(x1[cid * BPC:(cid + 1) * BPC])
        m["x2"] = np.ascontiguousarray(x2[cid * BPC:(cid + 1) * BPC])
        in_maps.append(m)

    import kernel as _self
    res = run_bass_kernel_spmd(nc, in_maps, core_ids=list(range(NCORES)),
                               trace=getattr(_self, "TRACE", False))
    _self.LAST_RESULT = res
    out = np.concatenate([np.asarray(r["out"]) for r in res.results], axis=0)
    return out.reshape(B, C, 128, 128).astype(np.float32)


TRACE = False
LAST_RESULT = None
